# revision 17
# baseline (speedup 1.0000x reference)
"""ChebNet (K=3) forward on 8 trn2 NeuronCores.

Math (reference):
    basis1 = [T0|T1|T2](x),  T0=x, T1=Lx, T2=2L(T1)-x       (L = sparse coo)
    h   = relu(basis1 @ W1 + b1)
    out = basis2(h) @ W2 + b2 ; return out[idx]

Restructured:
    pass1: T1 = L x                  (gather x_table, F=128 bf16)
    pass2: U  = L T1, fused dense:   h = relu(x A0 + T1 A1 + U A2 + b1)
           ab = h @ [W2_1 | 2 W2_2]  (node-major bf16 table [a | 2b])
           c  = h @ (W2_0 - W2_2) + b2
    pass3: acc = L [a|2b]; s = a_local + acc[:, 64:128]   (= a + 2Lb)
    pass4: out = c + L s, only at destinations in unique(idx).

Sharding: destinations split over 8 cores; per-core destinations are
greedily binned (100 bins x 128 slots) balancing per-source-block edge
counts so every (bin, src-block) cell fits exactly 8 chunks of 128
edges (zero padding). Passes 1-3 share one SBUF-resident index set
(same graph). SpMM = dma_gather of 256B rows + one-hot bf16 matmuls
accumulating per dest bin in PSUM. AllGathers rebuild full tables
between passes.
"""

import sys

sys.path.insert(0, "/opt/trn_rl_repo")

import numpy as np

import concourse.bass as bass
import concourse.tile as tile
import concourse.bass_utils as bass_utils
from concourse import bacc, mybir

NCORES = 8
P = 128
GS = 4
NSRC = 4


# --------------------------------------------------------------------------
# Host-side preprocessing
# --------------------------------------------------------------------------

def balance_bins(vecs, NB, cap=P):
    """Greedy multi-dim balanced binning: assign rows of vecs [n, NSRC]
    to NB bins (<=cap each) minimizing max per-coordinate bin sum."""
    n, k = vecs.shape
    order = np.argsort(-vecs.sum(1), kind="stable")
    V = np.zeros((NB, k), np.int64)
    cnt = np.zeros(NB, np.int64)
    assign = np.empty(n, np.int64)
    slot = np.empty(n, np.int64)
    BIG = 1 << 40
    for d in order:
        v = vecs[d]
        score = (V + v[None, :]).max(1) * 4096 + V.sum(1) // 8
        score[cnt >= cap] = BIG
        b = int(np.argmin(score))
        assign[d] = b
        slot[d] = cnt[b]
        V[b] += v
        cnt[b] += 1
    return assign, slot


def wrap_idx16(flat):
    n = len(flat)
    wrapped = flat.reshape(n // 16, 16).T
    return np.tile(wrapped, (8, 1)).astype(np.int16)


def build_call_structure(NB, Mtab):
    """calls: list of (s, [chunk ids]); chunk_meta[ch] = (b, s, m)."""
    calls = []
    chunk_meta = []
    first_chunk = {}
    ch = 0
    for sg0 in range(0, NB, GS):
        bins_sg = range(sg0, min(NB, sg0 + GS))
        for s in range(NSRC):
            cl = []
            for b in bins_sg:
                first_chunk[(b, s)] = ch
                for m in range(Mtab[b][s]):
                    chunk_meta.append((b, s, m))
                    cl.append(ch)
                    ch += 1
            if cl:
                calls.append((s, cl))
    return calls, chunk_meta, first_chunk, ch


def build_pass_tables(er_bin, er_slot, src_pos, ev, NB, Mtab, BS):
    calls, chunk_meta, first_chunk, NCH = build_call_structure(NB, Mtab)
    blk = src_pos // BS
    key = er_bin * NSRC + blk
    order_e = np.argsort(key, kind="stable")
    sk = key[order_e]
    cnt = np.bincount(sk, minlength=NB * NSRC)
    start = np.zeros(NB * NSRC + 1, np.int64)
    np.cumsum(cnt, out=start[1:])
    pos = np.arange(len(order_e)) - start[sk]
    fc = np.zeros(NB * NSRC, np.int64)
    for (b, s), c0 in first_chunk.items():
        fc[b * NSRC + s] = c0
    col = fc[sk] + pos // P
    lane = pos % P

    idxl = np.zeros((P, NCH), np.int16)
    dlocf = np.zeros((P, NCH), np.float32)
    valf = np.zeros((P, NCH), np.float32)
    idxl[lane, col] = (src_pos[order_e] % BS).astype(np.int16)
    dlocf[lane, col] = er_slot[order_e].astype(np.float32)
    valf[lane, col] = ev[order_e]

    idx16 = np.zeros((P, NCH * 8), np.int16)
    for s, cl in calls:
        n = len(cl) * P
        flat = idxl[:, cl[0]:cl[0] + len(cl)].T.reshape(n)
        idx16[:, cl[0] * 8:cl[0] * 8 + n // 16] = wrap_idx16(flat)
    return idx16, dlocf, valf, NCH


def idxA_local(idx16, cl):
    n = len(cl) * P
    w = idx16[0:16, cl[0] * 8:cl[0] * 8 + n // 16]
    flat = w.T.reshape(n).astype(np.int64)
    return flat.reshape(len(cl), P).T


def host_prep(x, vals, W1, b1, W2, b2, rows, cols, idx=None, K=3):
    import ml_dtypes
    bf16 = ml_dtypes.bfloat16
    N, F1 = x.shape
    FH = W1.shape[1]
    FO = W2.shape[1]
    SH = N // NCORES
    NB = 100
    SHP = NB * P
    NT = NCORES * SHP
    BS = NT // NSRC
    assert BS == 2 * SHP and BS <= 32767

    rows = np.asarray(rows)
    cols = np.asarray(cols)
    vals = np.asarray(vals, np.float32)
    x = np.asarray(x, np.float32)

    owner = rows // SH
    src_blk = (cols // SH) // 2
    core_edges = [np.nonzero(owner == c)[0] for c in range(NCORES)]

    # ---- pass A binning (all dests) ----
    remap2 = np.empty(N, np.int64)
    percore = []
    for c in range(NCORES):
        sel = core_edges[c]
        er = rows[sel] - c * SH
        keyv = er * NSRC + src_blk[sel]
        vec = np.bincount(keyv, minlength=SH * NSRC).reshape(SH, NSRC)
        assign, slot = balance_bins(vec, NB)
        jpos = assign * P + slot
        remap2[c * SH:(c + 1) * SH] = c * SHP + jpos
        percore.append(dict(sel=sel, er=er, assign=assign, slot=slot,
                            jpos=jpos))

    cntA = np.zeros((NCORES, NB, NSRC), np.int64)
    for c in range(NCORES):
        pc = percore[c]
        keyc = pc["assign"][pc["er"]] * NSRC + src_blk[pc["sel"]]
        cntA[c] = np.bincount(keyc, minlength=NB * NSRC).reshape(NB, NSRC)
    MtabA = np.ceil(cntA.max(0) / P).astype(np.int64)

    # ---- pass B binning (pruned dests) ----
    if idx is not None:
        uidx = np.unique(np.asarray(idx))
    else:
        uidx = np.arange(N)
    pruned = np.zeros(N, bool)
    pruned[uidx] = True
    n4s = []
    for c in range(NCORES):
        d4 = np.nonzero(pruned[c * SH:(c + 1) * SH])[0]
        percore[c]["d4"] = d4
        n4s.append(len(d4))
    NB4 = int(np.ceil(max(n4s) / P))
    for c in range(NCORES):
        pc = percore[c]
        d4 = pc["d4"]
        sel = pc["sel"]
        sel4 = sel[pruned[rows[sel]]]
        er4 = rows[sel4] - c * SH
        inv = -np.ones(SH, np.int64)
        inv[d4] = np.arange(len(d4))
        keyv = inv[er4] * NSRC + src_blk[sel4]
        vec = np.bincount(keyv, minlength=len(d4) * NSRC).reshape(-1, NSRC)
        a4, s4 = balance_bins(vec, NB4)
        pc["sel4"] = sel4
        pc["er4loc"] = inv[er4]
        pc["assign4"] = a4
        pc["slot4"] = s4
        pc["jpos4"] = a4 * P + s4
    cntB = np.zeros((NCORES, NB4, NSRC), np.int64)
    for c in range(NCORES):
        pc = percore[c]
        keyc = pc["assign4"][pc["er4loc"]] * NSRC + src_blk[pc["sel4"]]
        cntB[c] = np.bincount(keyc, minlength=NB4 * NSRC).reshape(NB4, NSRC)
    MtabB = np.ceil(cntB.max(0) / P).astype(np.int64)

    # ---- per-core tables ----
    cores = []
    for c in range(NCORES):
        pc = percore[c]
        sel, er = pc["sel"], pc["er"]
        idxA, dlocA, valA, NCHA = build_pass_tables(
            pc["assign"][er], pc["slot"][er], remap2[cols[sel]],
            vals[sel], NB, MtabA, BS)
        sel4 = pc["sel4"]
        e4l = pc["er4loc"]
        idxB, dlocB, valB, NCHB = build_pass_tables(
            pc["assign4"][e4l], pc["slot4"][e4l], remap2[cols[sel4]],
            vals[sel4], NB4, MtabB, BS)
        cidx_flat = np.zeros(NB4 * P, np.int16)
        cidx_flat[pc["jpos4"]] = pc["jpos"][pc["d4"]].astype(np.int16)
        cidx = wrap_idx16(cidx_flat)

        # pass-1 source rows pre-gathered into chunk-laned layout
        callsA_l, metaA_l, _, _ = build_call_structure(NB, MtabA)
        gidx = np.zeros((P, NCHA), np.int64)
        for s_, cl in callsA_l:
            base = s_ * BS
            gidx[:, cl[0]:cl[0] + len(cl)] = \
                base + idxA_local(idxA, cl)

        xT = np.zeros((F1, SHP), np.float32)
        xT[:, pc["jpos"]] = x[c * SH:(c + 1) * SH].T

        cores.append(dict(
            idxA=idxA, dlocA=dlocA, valA=valA, gidx=gidx,
            idxB=idxB, dlocB=dlocB, valB=valB,
            cidx=cidx, xTb=np.ascontiguousarray(xT.astype(bf16)),
            jpos=pc["jpos"], jpos4=pc["jpos4"], d4=pc["d4"],
        ))

    x_table = np.zeros((NT, F1), bf16)
    for c in range(NCORES):
        x_table[c * SHP + percore[c]["jpos"]] = \
            x[c * SH:(c + 1) * SH].astype(bf16)
    for c in range(NCORES):
        co = cores[c]
        co["xg"] = np.ascontiguousarray(x_table[co.pop("gidx")])

    W1r = np.asarray(W1, np.float32).reshape(F1, K, FH)
    A = np.stack([W1r[:, 0] - W1r[:, 2], W1r[:, 1], 2.0 * W1r[:, 2]])
    W2r = np.asarray(W2, np.float32).reshape(FH, K, FO)
    Cmat = np.concatenate(
        [W2r[:, 1], 2.0 * W2r[:, 2], W2r[:, 0] - W2r[:, 2]], axis=1)
    nkc = FH // P
    Ck = np.ascontiguousarray(Cmat.reshape(nkc, P, 3 * FO).astype(bf16))
    b1h = np.ascontiguousarray(np.asarray(b1, np.float32).reshape(nkc, P, 1))
    b2h = np.ascontiguousarray(np.asarray(b2, np.float32).reshape(FO, 1))
    iota = np.tile(np.arange(P), (P, 1)).astype(bf16)
    Ibf = np.eye(P, dtype=bf16)
    I64 = np.eye(FO, dtype=np.float32)

    shared = dict(x_table=x_table, A=np.ascontiguousarray(A.astype(bf16)),
                  Ck=Ck, b1h=b1h, b2h=b2h, iota=iota, Ibf=Ibf, I64=I64)
    dims = dict(N=N, F1=F1, FH=FH, FO=FO, SH=SH, NB=NB, SHP=SHP, NT=NT,
                BS=BS, NB4=NB4, NCHA=cores[0]["dlocA"].shape[1],
                NCHB=cores[0]["dlocB"].shape[1],
                MtabA=MtabA.tolist(), MtabB=MtabB.tolist())
    return dims, shared, cores


# --------------------------------------------------------------------------
# Device program
# --------------------------------------------------------------------------

def build_program(dims):
    d = dims
    F1, FH, FO = d["F1"], d["FH"], d["FO"]
    NB, SHP, NT = d["NB"], d["SHP"], d["NT"]
    BS, NB4 = d["BS"], d["NB4"]
    NCHA, NCHB = d["NCHA"], d["NCHB"]
    callsA, metaA, _, ncha = build_call_structure(NB, d["MtabA"])
    callsB, metaB, _, nchb = build_call_structure(NB4, d["MtabB"])
    assert ncha == NCHA and nchb == NCHB
    f32 = mybir.dt.float32
    i16 = mybir.dt.int16
    bf = mybir.dt.bfloat16
    nkc = FH // P

    # last chunk id per bin (for epilogue trigger)
    def last_chunks(meta):
        last = {}
        for ch, (b, s, m) in enumerate(meta):
            last[b] = ch
        return last
    lastA = last_chunks(metaA)
    lastB = last_chunks(metaB)
    firstA = {}
    firstB = {}
    for ch, (b, s, m) in enumerate(metaA):
        firstA.setdefault(b, ch)
    for ch, (b, s, m) in enumerate(metaB):
        firstB.setdefault(b, ch)

    nc = bacc.Bacc("TRN2", target_bir_lowering=False, debug=False,
                   num_devices=NCORES)

    # ---- I/O ----
    t_xg = nc.dram_tensor("xg", [P, NCHA, F1], bf, kind="ExternalInput")
    t_xT = nc.dram_tensor("xTb", [P, SHP], bf, kind="ExternalInput")
    t_idxA = nc.dram_tensor("idxA", [P, NCHA * 8], i16, kind="ExternalInput")
    t_dlocA = nc.dram_tensor("dlocA", [P, NCHA], f32, kind="ExternalInput")
    t_valA = nc.dram_tensor("valA", [P, NCHA], f32, kind="ExternalInput")
    t_idxB = nc.dram_tensor("idxB", [P, NCHB * 8], i16, kind="ExternalInput")
    t_dlocB = nc.dram_tensor("dlocB", [P, NCHB], f32, kind="ExternalInput")
    t_valB = nc.dram_tensor("valB", [P, NCHB], f32, kind="ExternalInput")
    t_cidx = nc.dram_tensor("cidx", [P, NB4 * 8], i16, kind="ExternalInput")
    t_A = nc.dram_tensor("A", [3, F1, FH], bf, kind="ExternalInput")
    t_Ck = nc.dram_tensor("Ck", [nkc, P, 3 * FO], bf, kind="ExternalInput")
    t_b1 = nc.dram_tensor("b1h", [nkc, P, 1], f32, kind="ExternalInput")
    t_b2 = nc.dram_tensor("b2h", [FO, 1], f32, kind="ExternalInput")
    t_iota = nc.dram_tensor("iota", [P, P], bf, kind="ExternalInput")
    t_Ibf = nc.dram_tensor("Ibf", [P, P], bf, kind="ExternalInput")
    t_I64 = nc.dram_tensor("I64", [FO, FO], f32, kind="ExternalInput")
    t_out = nc.dram_tensor("out2", [NB4 * P, FO], f32, kind="ExternalOutput")
    dbg = dims.get("debug", False)
    if dbg:
        t_dbg_t1 = nc.dram_tensor("dbg_t1", [SHP, F1], f32,
                                  kind="ExternalOutput")
        t_dbg_ab = nc.dram_tensor("dbg_ab", [SHP, F1], f32,
                                  kind="ExternalOutput")
        t_dbg_c = nc.dram_tensor("dbg_c", [SHP, FO], f32,
                                 kind="ExternalOutput")
        t_dbg_s = nc.dram_tensor("dbg_s", [SHP, FO], f32,
                                 kind="ExternalOutput")

    rg = [list(range(NCORES))]
    phases = dims.get("phases", 99)
    nocc = dims.get("nocc", False)

    def allgather(nc, src_t, dst_t):
        if nocc:
            nc.sync.dma_start(dst_t[0:src_t.shape[0], :], src_t[:])
        else:
            nc.gpsimd.collective_compute(
                "AllGather", mybir.AluOpType.bypass, replica_groups=rg,
                ins=[src_t.opt()], outs=[dst_t.opt()],
            )

    with tile.TileContext(nc) as tc:
        with (
            tc.tile_pool(name="const", bufs=1) as constp,
            tc.tile_pool(name="gat", bufs=3) as gatp,
            tc.tile_pool(name="sel", bufs=16) as selp,
            tc.tile_pool(name="stage", bufs=4) as stagep,
            tc.tile_pool(name="dense", bufs=4) as densep,
            tc.tile_pool(name="acc", bufs=6, space="PSUM") as accp,
            tc.tile_pool(name="dpsum", bufs=2, space="PSUM") as dpsump,
            tc.tile_pool(name="dram", bufs=1, space="DRAM") as dramp,
        ):
            # ---- internal DRAM ----
            t1b_in = dramp.tile([SHP, F1], bf, name="t1b_in")
            ab_in = dramp.tile([SHP, F1], bf, name="ab_in")
            c_in = dramp.tile([SHP, FO], f32, name="c_in")
            s_in = dramp.tile([SHP, F1], bf, name="s_in")
            t1_full = dramp.tile([NT, F1], bf, addr_space="Shared")
            ab_full = dramp.tile([NT, F1], bf, addr_space="Shared")
            s_full = dramp.tile([NT, F1], bf, addr_space="Shared")

            # ---- resident SBUF ----
            iota_sb = constp.tile([P, P], bf)
            nc.sync.dma_start(iota_sb[:], t_iota.ap())
            dlocA_sb = constp.tile([P, NCHA], f32)
            nc.sync.dma_start(dlocA_sb[:], t_dlocA.ap())
            valA_sb = constp.tile([P, NCHA], f32)
            nc.sync.dma_start(valA_sb[:], t_valA.ap())
            idxA_sb = constp.tile([P, NCHA * 8], i16)
            nc.sync.dma_start(idxA_sb[:], t_idxA.ap())
            Ibf_sb = constp.tile([P, P], bf)
            nc.sync.dma_start(Ibf_sb[:], t_Ibf.ap())
            xT_sb = constp.tile([P, SHP], bf)
            nc.sync.dma_start(xT_sb[:], t_xT.ap())
            A_sb = [constp.tile([F1, FH], bf, name=f"A{k}") for k in range(3)]
            for k in range(3):
                nc.sync.dma_start(A_sb[k][:], t_A.ap()[k])
            Ck_sb = [constp.tile([P, 3 * FO], bf, name=f"Ck{k}")
                     for k in range(nkc)]
            for k in range(nkc):
                nc.sync.dma_start(Ck_sb[k][:], t_Ck.ap()[k])
            b1_sb = [constp.tile([P, 1], f32, name=f"b1_{k}")
                     for k in range(nkc)]
            for k in range(nkc):
                nc.sync.dma_start(b1_sb[k][:], t_b1.ap()[k])
            b2_sb = constp.tile([FO, 1], f32)
            nc.sync.dma_start(b2_sb[:], t_b2.ap())
            I64_sb = constp.tile([FO, FO], f32)
            nc.sync.dma_start(I64_sb[:], t_I64.ap())
            dlocB_sb = constp.tile([P, NCHB], f32)
            nc.sync.dma_start(dlocB_sb[:], t_dlocB.ap())
            valB_sb = constp.tile([P, NCHB], f32)
            nc.sync.dma_start(valB_sb[:], t_valB.ap())
            cidx_sb = constp.tile([P, NB4 * 8], i16)
            nc.sync.dma_start(cidx_sb[:], t_cidx.ap())
            # residents written by compute
            t1T_sb = constp.tile([P, SHP], bf, name="t1T")
            cg_sb = constp.tile([P, NB4, FO], f32, name="cg_sb")

            # ---------------- generic spmm pass ----------------
            def spmm(tag, table_ap, F, epilogue, calls, meta, idx_sb,
                     dloc_sb, val_sb, first, last, idx_dram=None,
                     src_copy=None, s_engines=None):
                if s_engines is None:
                    s_engines = [nc.vector]
                alive = {}
                s_eng_i = 0
                for s, cl in calls:
                    n_idx = len(cl) * P
                    c0 = cl[0] * 8
                    if src_copy is not None:
                        g = gatp.tile([P, len(cl), F1], bf,
                                      name=f"g_{tag}", tag="g")
                        nc.sync.dma_start(
                            g[:], src_copy.ap()[:, cl[0]:cl[0] + len(cl), :])
                    elif idx_dram is not None:
                        idx_t = selp.tile([P, n_idx // 16], i16,
                                          name=f"idx_{tag}", tag="idxt")
                        nc.sync.dma_start(idx_t[:],
                                          idx_dram.ap()[:, c0:c0 + n_idx // 16])
                        idx_ap = idx_t[:]
                    else:
                        idx_ap = idx_sb[:, c0:c0 + n_idx // 16]
                    if src_copy is None:
                        g = gatp.tile([P, len(cl), F1], bf, name=f"g_{tag}",
                                      tag="g")
                        nc.gpsimd.dma_gather(
                            out_ap=g[:],
                            in_ap=table_ap[s * BS:(s + 1) * BS, :],
                            idxs_ap=idx_ap,
                            num_idxs=n_idx, num_idxs_reg=n_idx,
                            elem_size=F1, single_packet=False,
                        )
                    for j, ch in enumerate(cl):
                        b, src, m = meta[ch]
                        if first[b] == ch:
                            alive[b] = accp.tile([P, F], f32,
                                                 name=f"acc_{tag}",
                                                 tag="acc", space="PSUM")
                        acc = alive[b]
                        S = selp.tile([P, P], bf, name=f"S_{tag}", tag="S")
                        eng = s_engines[s_eng_i % len(s_engines)]
                        s_eng_i += 1
                        eng.tensor_scalar(
                            out=S[:], in0=iota_sb[:],
                            scalar1=dloc_sb[:, ch:ch + 1],
                            scalar2=val_sb[:, ch:ch + 1],
                            op0=mybir.AluOpType.is_equal,
                            op1=mybir.AluOpType.mult,
                        )
                        nc.tensor.matmul(
                            out=acc[:], lhsT=S[:], rhs=g[:, j, 0:F],
                            start=(first[b] == ch), stop=(last[b] == ch),
                        )
                        if last[b] == ch:
                            epilogue(b, acc)
                            del alive[b]

            # ---------------- pass 1: T1 = L x ----------------
            def epi1(b, acc):
                stb = stagep.tile([P, F1], bf, name="st1", tag="st1")
                nc.scalar.activation(stb[:], acc[:],
                                     mybir.ActivationFunctionType.Copy)
                nc.scalar.dma_start(t1b_in[b * P:(b + 1) * P, :], stb[:])
                pt = dpsump.tile([P, P], bf, tag="dps", space="PSUM")
                nc.tensor.transpose(pt[:], stb[:], Ibf_sb[:])
                nc.vector.tensor_copy(t1T_sb[:, b * P:(b + 1) * P], pt[:])

            with nc.named_scope("pass1"):
                spmm("t1", None, F1, epi1, callsA, metaA, idxA_sb,
                     dlocA_sb, valA_sb, firstA, lastA, src_copy=t_xg,
                     s_engines=[nc.vector, nc.vector, nc.gpsimd])

            if phases >= 2:
                with nc.named_scope("ag_t1"):
                    allgather(nc, t1b_in, t1_full)

            # ------- pass 2: U = L T1, fused dense -------
            def epi2(b, acc):
                ns = slice(b * P, (b + 1) * P)
                ub = stagep.tile([P, F1], bf, name="ub", tag="ub")
                nc.scalar.activation(ub[:], acc[:],
                                     mybir.ActivationFunctionType.Copy)
                ptu = dpsump.tile([P, P], bf, tag="dps", space="PSUM")
                nc.tensor.transpose(ptu[:], ub[:], Ibf_sb[:])
                uT = densep.tile([P, P], bf, tag="uT")
                nc.vector.tensor_copy(uT[:], ptu[:])
                hT = []
                for half in range(nkc):
                    cs = slice(half * P, (half + 1) * P)
                    ph = dpsump.tile([P, P], f32, tag="dps", space="PSUM")
                    nc.tensor.matmul(ph[:], lhsT=A_sb[0][:, cs],
                                     rhs=xT_sb[:, ns], start=True, stop=False)
                    nc.tensor.matmul(ph[:], lhsT=A_sb[1][:, cs],
                                     rhs=t1T_sb[:, ns], start=False,
                                     stop=False)
                    nc.tensor.matmul(ph[:], lhsT=A_sb[2][:, cs],
                                     rhs=uT[:], start=False, stop=True)
                    h = densep.tile([P, P], bf, name=f"hT{half}",
                                    tag=f"hT{half}")
                    nc.scalar.activation(h[:], ph[:],
                                         mybir.ActivationFunctionType.Relu,
                                         bias=b1_sb[half][:])
                    hT.append(h)
                # ab = [a | 2b] feature-major -> node-major
                pab = dpsump.tile([P, P], f32, tag="dps", space="PSUM")
                for k in range(nkc):
                    nc.tensor.matmul(pab[:], lhsT=Ck_sb[k][:, 0:2 * FO],
                                     rhs=hT[k][:], start=(k == 0),
                                     stop=(k == nkc - 1))
                abf = stagep.tile([P, P], bf, name="abf", tag="abf")
                nc.scalar.activation(abf[:], pab[:],
                                     mybir.ActivationFunctionType.Copy)
                pta = dpsump.tile([P, P], bf, tag="dps", space="PSUM")
                nc.tensor.transpose(pta[:], abf[:], Ibf_sb[:])
                abn = stagep.tile([P, P], bf, name="abn", tag="abn")
                nc.vector.tensor_copy(abn[:], pta[:])
                nc.scalar.dma_start(ab_in[ns, :], abn[:])
                # c = h @ (W2_0 - W2_2) + b2
                pc_ = dpsump.tile([FO, P], f32, tag="dps", space="PSUM")
                for k in range(nkc):
                    nc.tensor.matmul(pc_[:], lhsT=Ck_sb[k][:, 2 * FO:3 * FO],
                                     rhs=hT[k][:], start=(k == 0),
                                     stop=(k == nkc - 1))
                cf = densep.tile([FO, P], f32, tag="cf")
                nc.vector.tensor_scalar(out=cf[:], in0=pc_[:],
                                        scalar1=b2_sb[:], scalar2=None,
                                        op0=mybir.AluOpType.add)
                ptc = dpsump.tile([P, FO], f32, tag="dps", space="PSUM")
                nc.tensor.transpose(ptc[:], cf[:], I64_sb[:])
                cn = densep.tile([P, FO], f32, tag="cn")
                nc.vector.tensor_copy(cn[:], ptc[:])
                nc.scalar.dma_start(c_in[ns, :], cn[:])

            if phases >= 3:
                with nc.named_scope("pass2"):
                    spmm("u", t1_full[:], F1, epi2, callsA, metaA, idxA_sb,
                         dlocA_sb, valA_sb, firstA, lastA,
                         s_engines=[nc.vector] * 5 + [nc.gpsimd])

            if phases >= 4:
                with nc.named_scope("ag_ab"):
                    allgather(nc, ab_in, ab_full)
                # gather c rows into pass-4 slot order (overlaps pass 3)
                nc.gpsimd.dma_gather(
                    out_ap=cg_sb[:], in_ap=c_in[:],
                    idxs_ap=cidx_sb[:, 0:NB4 * 8],
                    num_idxs=NB4 * P, num_idxs_reg=NB4 * P,
                    elem_size=FO, single_packet=False,
                )

            # ------- pass 3: s = a + 2Lb -------
            def epi3(b, acc):
                asb = stagep.tile([P, FO], bf, name="asb", tag="asb")
                nc.sync.dma_start(asb[:], ab_in[b * P:(b + 1) * P, 0:FO])
                st = stagep.tile([P, FO], bf, name="st3", tag="st3")
                nc.vector.tensor_tensor(out=st[:], in0=acc[:, FO:2 * FO],
                                        in1=asb[:],
                                        op=mybir.AluOpType.add)
                nc.scalar.dma_start(s_in[b * P:(b + 1) * P, 0:FO], st[:])

            if phases >= 4:
                with nc.named_scope("pass3"):
                    spmm("s", ab_full[:], F1, epi3, callsA, metaA, idxA_sb,
                         dlocA_sb, valA_sb, firstA, lastA,
                         s_engines=[nc.vector] * 5 + [nc.gpsimd])

            if phases >= 5:
                with nc.named_scope("ag_s"):
                    allgather(nc, s_in, s_full)

            # ------- pass 4: out = c + L s (pruned) -------
            def epi4(b, acc):
                st = stagep.tile([P, FO], f32, name="st4", tag="st4")
                nc.vector.tensor_tensor(out=st[:], in0=acc[:],
                                        in1=cg_sb[:, b, :],
                                        op=mybir.AluOpType.add)
                nc.scalar.dma_start(t_out.ap()[b * P:(b + 1) * P, :], st[:])

            if phases >= 5:
                with nc.named_scope("pass4"):
                    spmm("o", s_full[:], FO, epi4, callsB, metaB, None,
                         dlocB_sb, valB_sb, firstB, lastB, idx_dram=t_idxB,
                         s_engines=[nc.vector] * 5 + [nc.gpsimd])

            if dbg:
                def cvt(dst, src, F):
                    for t in range(0, SHP, P):
                        tl = stagep.tile([P, F], f32, tag=f"dbg{F}")
                        sb = stagep.tile([P, F], bf, tag=f"dbgb{F}")
                        nc.sync.dma_start(sb[:], src[t:t + P, 0:F])
                        nc.vector.tensor_copy(tl[:], sb[:])
                        nc.sync.dma_start(dst[t:t + P, 0:F], tl[:])
                cvt(t_dbg_t1.ap(), t1b_in, F1)
                if phases >= 3:
                    cvt(t_dbg_ab.ap(), ab_in, F1)
                    nc.sync.dma_start(t_dbg_c.ap(), c_in[:])
                if phases >= 4:
                    cvt(t_dbg_s.ap(), s_in, FO)

    nc.compile()
    return nc


# --------------------------------------------------------------------------
# Entry point
# --------------------------------------------------------------------------

def run(x, vals, W1, b1, W2, b2, rows, cols, idx=None, trace=False,
        debug=False):
    dims, shared, cores = host_prep(x, vals, W1, b1, W2, b2, rows, cols,
                                    idx=idx)
    if debug:
        dims["debug"] = True
    nc = build_program(dims)
    in_maps = []
    for c in range(NCORES):
        m = dict(
            A=shared["A"], Ck=shared["Ck"],
            b1h=shared["b1h"], b2h=shared["b2h"], iota=shared["iota"],
            Ibf=shared["Ibf"], I64=shared["I64"],
            xTb=cores[c]["xTb"], idxA=cores[c]["idxA"],
            xg=cores[c]["xg"],
            dlocA=cores[c]["dlocA"], valA=cores[c]["valA"],
            idxB=cores[c]["idxB"], dlocB=cores[c]["dlocB"],
            valB=cores[c]["valB"], cidx=cores[c]["cidx"],
        )
        in_maps.append(m)
    try:
        res = bass_utils.run_bass_kernel_spmd(
            nc, in_maps, core_ids=list(range(NCORES)), trace=trace)
    except Exception:
        res = bass_utils.run_bass_kernel_spmd(
            nc, in_maps, core_ids=list(range(NCORES)), trace=trace)
    SH = dims["SH"]
    N = dims["N"]
    FO = dims["FO"]
    out_full = np.zeros((N, FO), np.float32)
    for c in range(NCORES):
        co = cores[c]
        out_full[c * SH + co["d4"]] = res.results[c]["out2"][co["jpos4"]]
    return out_full, res


def kernel(x, vals, W1, b1, W2, b2, rows, cols, idx):
    out_full, _ = run(np.asarray(x), np.asarray(vals), np.asarray(W1),
                      np.asarray(b1), np.asarray(W2), np.asarray(b2),
                      np.asarray(rows), np.asarray(cols),
                      idx=np.asarray(idx))
    return out_full[np.asarray(idx)]


# revision 18
# speedup vs baseline: 1.0077x; 1.0077x over previous
"""ChebNet (K=3) forward on 8 trn2 NeuronCores.

Math (reference):
    basis1 = [T0|T1|T2](x),  T0=x, T1=Lx, T2=2L(T1)-x       (L = sparse coo)
    h   = relu(basis1 @ W1 + b1)
    out = basis2(h) @ W2 + b2 ; return out[idx]

Restructured:
    pass1: T1 = L x                  (gather x_table, F=128 bf16)
    pass2: U  = L T1, fused dense:   h = relu(x A0 + T1 A1 + U A2 + b1)
           ab = h @ [W2_1 | 2 W2_2]  (node-major bf16 table [a | 2b])
           c  = h @ (W2_0 - W2_2) + b2
    pass3: acc = L [a|2b]; s = a_local + acc[:, 64:128]   (= a + 2Lb)
    pass4: out = c + L s, only at destinations in unique(idx).

Sharding: destinations split over 8 cores; per-core destinations are
greedily binned (100 bins x 128 slots) balancing per-source-block edge
counts so every (bin, src-block) cell fits exactly 8 chunks of 128
edges (zero padding). Passes 1-3 share one SBUF-resident index set
(same graph). SpMM = dma_gather of 256B rows + one-hot bf16 matmuls
accumulating per dest bin in PSUM. AllGathers rebuild full tables
between passes.
"""

import sys

sys.path.insert(0, "/opt/trn_rl_repo")

import numpy as np

import concourse.bass as bass
import concourse.tile as tile
import concourse.bass_utils as bass_utils
from concourse import bacc, mybir

NCORES = 8
P = 128
GS = 4
NSRC = 4


# --------------------------------------------------------------------------
# Host-side preprocessing
# --------------------------------------------------------------------------

def balance_bins(vecs, NB, cap=P):
    """Greedy multi-dim balanced binning: assign rows of vecs [n, NSRC]
    to NB bins (<=cap each) minimizing max per-coordinate bin sum."""
    n, k = vecs.shape
    order = np.argsort(-vecs.sum(1), kind="stable")
    V = np.zeros((NB, k), np.int64)
    cnt = np.zeros(NB, np.int64)
    assign = np.empty(n, np.int64)
    slot = np.empty(n, np.int64)
    BIG = 1 << 40
    for d in order:
        v = vecs[d]
        score = (V + v[None, :]).max(1) * 4096 + V.sum(1) // 8
        score[cnt >= cap] = BIG
        b = int(np.argmin(score))
        assign[d] = b
        slot[d] = cnt[b]
        V[b] += v
        cnt[b] += 1
    return assign, slot


def wrap_idx16(flat):
    n = len(flat)
    wrapped = flat.reshape(n // 16, 16).T
    return np.tile(wrapped, (8, 1)).astype(np.int16)


def build_call_structure(NB, Mtab):
    """calls: list of (s, [chunk ids]); chunk_meta[ch] = (b, s, m)."""
    calls = []
    chunk_meta = []
    first_chunk = {}
    ch = 0
    for sg0 in range(0, NB, GS):
        bins_sg = range(sg0, min(NB, sg0 + GS))
        for s in range(NSRC):
            cl = []
            for b in bins_sg:
                first_chunk[(b, s)] = ch
                for m in range(Mtab[b][s]):
                    chunk_meta.append((b, s, m))
                    cl.append(ch)
                    ch += 1
            if cl:
                calls.append((s, cl))
    return calls, chunk_meta, first_chunk, ch


def build_pass_tables(er_bin, er_slot, src_pos, ev, NB, Mtab, BS):
    calls, chunk_meta, first_chunk, NCH = build_call_structure(NB, Mtab)
    blk = src_pos // BS
    key = er_bin * NSRC + blk
    order_e = np.argsort(key, kind="stable")
    sk = key[order_e]
    cnt = np.bincount(sk, minlength=NB * NSRC)
    start = np.zeros(NB * NSRC + 1, np.int64)
    np.cumsum(cnt, out=start[1:])
    pos = np.arange(len(order_e)) - start[sk]
    fc = np.zeros(NB * NSRC, np.int64)
    for (b, s), c0 in first_chunk.items():
        fc[b * NSRC + s] = c0
    col = fc[sk] + pos // P
    lane = pos % P

    idxl = np.zeros((P, NCH), np.int16)
    dlocf = np.zeros((P, NCH), np.float32)
    valf = np.zeros((P, NCH), np.float32)
    idxl[lane, col] = (src_pos[order_e] % BS).astype(np.int16)
    dlocf[lane, col] = er_slot[order_e].astype(np.float32)
    valf[lane, col] = ev[order_e]

    idx16 = np.zeros((P, NCH * 8), np.int16)
    for s, cl in calls:
        n = len(cl) * P
        flat = idxl[:, cl[0]:cl[0] + len(cl)].T.reshape(n)
        idx16[:, cl[0] * 8:cl[0] * 8 + n // 16] = wrap_idx16(flat)
    return idx16, dlocf, valf, NCH


def idxA_local(idx16, cl):
    n = len(cl) * P
    w = idx16[0:16, cl[0] * 8:cl[0] * 8 + n // 16]
    flat = w.T.reshape(n).astype(np.int64)
    return flat.reshape(len(cl), P).T


def host_prep(x, vals, W1, b1, W2, b2, rows, cols, idx=None, K=3):
    import ml_dtypes
    bf16 = ml_dtypes.bfloat16
    N, F1 = x.shape
    FH = W1.shape[1]
    FO = W2.shape[1]
    SH = N // NCORES
    NB = 100
    SHP = NB * P
    NT = NCORES * SHP
    BS = NT // NSRC
    assert BS == 2 * SHP and BS <= 32767

    rows = np.asarray(rows)
    cols = np.asarray(cols)
    vals = np.asarray(vals, np.float32)
    x = np.asarray(x, np.float32)

    owner = rows // SH
    src_blk = (cols // SH) // 2
    core_edges = [np.nonzero(owner == c)[0] for c in range(NCORES)]

    # ---- pass A binning (all dests) ----
    remap2 = np.empty(N, np.int64)
    percore = []
    for c in range(NCORES):
        sel = core_edges[c]
        er = rows[sel] - c * SH
        keyv = er * NSRC + src_blk[sel]
        vec = np.bincount(keyv, minlength=SH * NSRC).reshape(SH, NSRC)
        assign, slot = balance_bins(vec, NB)
        jpos = assign * P + slot
        remap2[c * SH:(c + 1) * SH] = c * SHP + jpos
        percore.append(dict(sel=sel, er=er, assign=assign, slot=slot,
                            jpos=jpos))

    cntA = np.zeros((NCORES, NB, NSRC), np.int64)
    for c in range(NCORES):
        pc = percore[c]
        keyc = pc["assign"][pc["er"]] * NSRC + src_blk[pc["sel"]]
        cntA[c] = np.bincount(keyc, minlength=NB * NSRC).reshape(NB, NSRC)
    MtabA = np.ceil(cntA.max(0) / P).astype(np.int64)

    # ---- pass B binning (pruned dests) ----
    if idx is not None:
        uidx = np.unique(np.asarray(idx))
    else:
        uidx = np.arange(N)
    pruned = np.zeros(N, bool)
    pruned[uidx] = True
    n4s = []
    for c in range(NCORES):
        d4 = np.nonzero(pruned[c * SH:(c + 1) * SH])[0]
        percore[c]["d4"] = d4
        n4s.append(len(d4))
    NB4 = int(np.ceil(max(n4s) / P))
    for c in range(NCORES):
        pc = percore[c]
        d4 = pc["d4"]
        sel = pc["sel"]
        sel4 = sel[pruned[rows[sel]]]
        er4 = rows[sel4] - c * SH
        inv = -np.ones(SH, np.int64)
        inv[d4] = np.arange(len(d4))
        keyv = inv[er4] * NSRC + src_blk[sel4]
        vec = np.bincount(keyv, minlength=len(d4) * NSRC).reshape(-1, NSRC)
        a4, s4 = balance_bins(vec, NB4)
        pc["sel4"] = sel4
        pc["er4loc"] = inv[er4]
        pc["assign4"] = a4
        pc["slot4"] = s4
        pc["jpos4"] = a4 * P + s4
    cntB = np.zeros((NCORES, NB4, NSRC), np.int64)
    for c in range(NCORES):
        pc = percore[c]
        keyc = pc["assign4"][pc["er4loc"]] * NSRC + src_blk[pc["sel4"]]
        cntB[c] = np.bincount(keyc, minlength=NB4 * NSRC).reshape(NB4, NSRC)
    MtabB = np.ceil(cntB.max(0) / P).astype(np.int64)

    # ---- per-core tables ----
    cores = []
    for c in range(NCORES):
        pc = percore[c]
        sel, er = pc["sel"], pc["er"]
        idxA, dlocA, valA, NCHA = build_pass_tables(
            pc["assign"][er], pc["slot"][er], remap2[cols[sel]],
            vals[sel], NB, MtabA, BS)
        sel4 = pc["sel4"]
        e4l = pc["er4loc"]
        idxB, dlocB, valB, NCHB = build_pass_tables(
            pc["assign4"][e4l], pc["slot4"][e4l], remap2[cols[sel4]],
            vals[sel4], NB4, MtabB, BS)
        cidx_flat = np.zeros(NB4 * P, np.int16)
        cidx_flat[pc["jpos4"]] = pc["jpos"][pc["d4"]].astype(np.int16)
        cidx = wrap_idx16(cidx_flat)

        # pass-1 source rows pre-gathered into chunk-laned layout
        callsA_l, metaA_l, _, _ = build_call_structure(NB, MtabA)
        gidx = np.zeros((P, NCHA), np.int64)
        for s_, cl in callsA_l:
            base = s_ * BS
            gidx[:, cl[0]:cl[0] + len(cl)] = \
                base + idxA_local(idxA, cl)

        xT = np.zeros((F1, SHP), np.float32)
        xT[:, pc["jpos"]] = x[c * SH:(c + 1) * SH].T

        cores.append(dict(
            idxA=idxA, dlocA=dlocA, valA=valA, gidx=gidx,
            idxB=idxB, dlocB=dlocB, valB=valB,
            cidx=cidx, xTb=np.ascontiguousarray(xT.astype(bf16)),
            jpos=pc["jpos"], jpos4=pc["jpos4"], d4=pc["d4"],
        ))

    x_table = np.zeros((NT, F1), bf16)
    for c in range(NCORES):
        x_table[c * SHP + percore[c]["jpos"]] = \
            x[c * SH:(c + 1) * SH].astype(bf16)
    for c in range(NCORES):
        co = cores[c]
        co["xg"] = np.ascontiguousarray(x_table[co.pop("gidx")])

    W1r = np.asarray(W1, np.float32).reshape(F1, K, FH)
    A = np.stack([W1r[:, 0] - W1r[:, 2], W1r[:, 1], 2.0 * W1r[:, 2]])
    W2r = np.asarray(W2, np.float32).reshape(FH, K, FO)
    Cmat = np.concatenate(
        [W2r[:, 1], 2.0 * W2r[:, 2], W2r[:, 0] - W2r[:, 2]], axis=1)
    nkc = FH // P
    Ck = np.ascontiguousarray(Cmat.reshape(nkc, P, 3 * FO).astype(bf16))
    b1h = np.ascontiguousarray(np.asarray(b1, np.float32).reshape(nkc, P, 1))
    b2h = np.ascontiguousarray(np.asarray(b2, np.float32).reshape(FO, 1))
    iota = np.tile(np.arange(P), (P, 1)).astype(bf16)
    Ibf = np.eye(P, dtype=bf16)
    I64 = np.eye(FO, dtype=np.float32)

    shared = dict(x_table=x_table, A=np.ascontiguousarray(A.astype(bf16)),
                  Ck=Ck, b1h=b1h, b2h=b2h, iota=iota, Ibf=Ibf, I64=I64)
    dims = dict(N=N, F1=F1, FH=FH, FO=FO, SH=SH, NB=NB, SHP=SHP, NT=NT,
                BS=BS, NB4=NB4, NCHA=cores[0]["dlocA"].shape[1],
                NCHB=cores[0]["dlocB"].shape[1],
                MtabA=MtabA.tolist(), MtabB=MtabB.tolist())
    return dims, shared, cores


# --------------------------------------------------------------------------
# Device program
# --------------------------------------------------------------------------

def build_program(dims):
    d = dims
    F1, FH, FO = d["F1"], d["FH"], d["FO"]
    NB, SHP, NT = d["NB"], d["SHP"], d["NT"]
    BS, NB4 = d["BS"], d["NB4"]
    NCHA, NCHB = d["NCHA"], d["NCHB"]
    callsA, metaA, _, ncha = build_call_structure(NB, d["MtabA"])
    callsB, metaB, _, nchb = build_call_structure(NB4, d["MtabB"])
    assert ncha == NCHA and nchb == NCHB
    f32 = mybir.dt.float32
    i16 = mybir.dt.int16
    bf = mybir.dt.bfloat16
    nkc = FH // P

    # last chunk id per bin (for epilogue trigger)
    def last_chunks(meta):
        last = {}
        for ch, (b, s, m) in enumerate(meta):
            last[b] = ch
        return last
    lastA = last_chunks(metaA)
    lastB = last_chunks(metaB)
    firstA = {}
    firstB = {}
    for ch, (b, s, m) in enumerate(metaA):
        firstA.setdefault(b, ch)
    for ch, (b, s, m) in enumerate(metaB):
        firstB.setdefault(b, ch)

    nc = bacc.Bacc("TRN2", target_bir_lowering=False, debug=False,
                   num_devices=NCORES)

    # ---- I/O ----
    t_xg = nc.dram_tensor("xg", [P, NCHA, F1], bf, kind="ExternalInput")
    t_xT = nc.dram_tensor("xTb", [P, SHP], bf, kind="ExternalInput")
    t_idxA = nc.dram_tensor("idxA", [P, NCHA * 8], i16, kind="ExternalInput")
    t_dlocA = nc.dram_tensor("dlocA", [P, NCHA], f32, kind="ExternalInput")
    t_valA = nc.dram_tensor("valA", [P, NCHA], f32, kind="ExternalInput")
    t_idxB = nc.dram_tensor("idxB", [P, NCHB * 8], i16, kind="ExternalInput")
    t_dlocB = nc.dram_tensor("dlocB", [P, NCHB], f32, kind="ExternalInput")
    t_valB = nc.dram_tensor("valB", [P, NCHB], f32, kind="ExternalInput")
    t_cidx = nc.dram_tensor("cidx", [P, NB4 * 8], i16, kind="ExternalInput")
    t_A = nc.dram_tensor("A", [3, F1, FH], bf, kind="ExternalInput")
    t_Ck = nc.dram_tensor("Ck", [nkc, P, 3 * FO], bf, kind="ExternalInput")
    t_b1 = nc.dram_tensor("b1h", [nkc, P, 1], f32, kind="ExternalInput")
    t_b2 = nc.dram_tensor("b2h", [FO, 1], f32, kind="ExternalInput")
    t_iota = nc.dram_tensor("iota", [P, P], bf, kind="ExternalInput")
    t_Ibf = nc.dram_tensor("Ibf", [P, P], bf, kind="ExternalInput")
    t_I64 = nc.dram_tensor("I64", [FO, FO], f32, kind="ExternalInput")
    t_out = nc.dram_tensor("out2", [NB4 * P, FO], f32, kind="ExternalOutput")
    dbg = dims.get("debug", False)
    if dbg:
        t_dbg_t1 = nc.dram_tensor("dbg_t1", [SHP, F1], f32,
                                  kind="ExternalOutput")
        t_dbg_ab = nc.dram_tensor("dbg_ab", [SHP, F1], f32,
                                  kind="ExternalOutput")
        t_dbg_c = nc.dram_tensor("dbg_c", [SHP, FO], f32,
                                 kind="ExternalOutput")
        t_dbg_s = nc.dram_tensor("dbg_s", [SHP, FO], f32,
                                 kind="ExternalOutput")

    rg = [list(range(NCORES))]
    phases = dims.get("phases", 99)
    nocc = dims.get("nocc", False)

    def allgather(nc, src_t, dst_t):
        if nocc:
            nc.sync.dma_start(dst_t[0:src_t.shape[0], :], src_t[:])
        else:
            nc.gpsimd.collective_compute(
                "AllGather", mybir.AluOpType.bypass, replica_groups=rg,
                ins=[src_t.opt()], outs=[dst_t.opt()],
            )

    with tile.TileContext(nc) as tc:
        with (
            tc.tile_pool(name="const", bufs=1) as constp,
            tc.tile_pool(name="gat", bufs=3) as gatp,
            tc.tile_pool(name="sel", bufs=20) as selp,
            tc.tile_pool(name="stage", bufs=4) as stagep,
            tc.tile_pool(name="dense", bufs=4) as densep,
            tc.tile_pool(name="acc", bufs=5, space="PSUM") as accp,
            tc.tile_pool(name="dpsum", bufs=3, space="PSUM") as dpsump,
            tc.tile_pool(name="dram", bufs=1, space="DRAM") as dramp,
        ):
            # ---- internal DRAM ----
            t1b_in = dramp.tile([SHP, F1], bf, name="t1b_in")
            ab_in = dramp.tile([SHP, F1], bf, name="ab_in")
            c_in = dramp.tile([SHP, FO], f32, name="c_in")
            s_in = dramp.tile([SHP, F1], bf, name="s_in")
            t1_full = dramp.tile([NT, F1], bf, addr_space="Shared")
            ab_full = dramp.tile([NT, F1], bf, addr_space="Shared")
            s_full = dramp.tile([NT, F1], bf, addr_space="Shared")

            # ---- resident SBUF ----
            iota_sb = constp.tile([P, P], bf)
            nc.sync.dma_start(iota_sb[:], t_iota.ap())
            dlocA_sb = constp.tile([P, NCHA], f32)
            nc.sync.dma_start(dlocA_sb[:], t_dlocA.ap())
            valA_sb = constp.tile([P, NCHA], f32)
            nc.sync.dma_start(valA_sb[:], t_valA.ap())
            idxA_sb = constp.tile([P, NCHA * 8], i16)
            nc.sync.dma_start(idxA_sb[:], t_idxA.ap())
            Ibf_sb = constp.tile([P, P], bf)
            nc.sync.dma_start(Ibf_sb[:], t_Ibf.ap())
            xT_sb = constp.tile([P, SHP], bf)
            nc.sync.dma_start(xT_sb[:], t_xT.ap())
            A_sb = [constp.tile([F1, FH], bf, name=f"A{k}") for k in range(3)]
            for k in range(3):
                nc.sync.dma_start(A_sb[k][:], t_A.ap()[k])
            Ck_sb = [constp.tile([P, 3 * FO], bf, name=f"Ck{k}")
                     for k in range(nkc)]
            for k in range(nkc):
                nc.sync.dma_start(Ck_sb[k][:], t_Ck.ap()[k])
            b1_sb = [constp.tile([P, 1], f32, name=f"b1_{k}")
                     for k in range(nkc)]
            for k in range(nkc):
                nc.sync.dma_start(b1_sb[k][:], t_b1.ap()[k])
            b2_sb = constp.tile([FO, 1], f32)
            nc.sync.dma_start(b2_sb[:], t_b2.ap())
            I64_sb = constp.tile([FO, FO], f32)
            nc.sync.dma_start(I64_sb[:], t_I64.ap())
            dlocB_sb = constp.tile([P, NCHB], f32)
            nc.sync.dma_start(dlocB_sb[:], t_dlocB.ap())
            valB_sb = constp.tile([P, NCHB], f32)
            nc.sync.dma_start(valB_sb[:], t_valB.ap())
            cidx_sb = constp.tile([P, NB4 * 8], i16)
            nc.sync.dma_start(cidx_sb[:], t_cidx.ap())
            # residents written by compute
            t1T_sb = constp.tile([P, SHP], bf, name="t1T")
            cg_sb = constp.tile([P, NB4, FO], f32, name="cg_sb")

            # ---------------- generic spmm pass ----------------
            def spmm(tag, table_ap, F, epilogue, calls, meta, idx_sb,
                     dloc_sb, val_sb, first, last, idx_dram=None,
                     src_copy=None, s_engines=None):
                if s_engines is None:
                    s_engines = [nc.vector]
                alive = {}
                s_eng_i = 0
                for s, cl in calls:
                    n_idx = len(cl) * P
                    c0 = cl[0] * 8
                    if src_copy is not None:
                        g = gatp.tile([P, len(cl), F1], bf,
                                      name=f"g_{tag}", tag="g")
                        nc.sync.dma_start(
                            g[:], src_copy.ap()[:, cl[0]:cl[0] + len(cl), :])
                    elif idx_dram is not None:
                        idx_t = selp.tile([P, n_idx // 16], i16,
                                          name=f"idx_{tag}", tag="idxt")
                        nc.sync.dma_start(idx_t[:],
                                          idx_dram.ap()[:, c0:c0 + n_idx // 16])
                        idx_ap = idx_t[:]
                    else:
                        idx_ap = idx_sb[:, c0:c0 + n_idx // 16]
                    if src_copy is None:
                        g = gatp.tile([P, len(cl), F1], bf, name=f"g_{tag}",
                                      tag="g")
                        nc.gpsimd.dma_gather(
                            out_ap=g[:],
                            in_ap=table_ap[s * BS:(s + 1) * BS, :],
                            idxs_ap=idx_ap,
                            num_idxs=n_idx, num_idxs_reg=n_idx,
                            elem_size=F1, single_packet=False,
                        )
                    for j, ch in enumerate(cl):
                        b, src, m = meta[ch]
                        if first[b] == ch:
                            alive[b] = accp.tile([P, F], f32,
                                                 name=f"acc_{tag}",
                                                 tag="acc", space="PSUM")
                        acc = alive[b]
                        S = selp.tile([P, P], bf, name=f"S_{tag}", tag="S")
                        eng = s_engines[s_eng_i % len(s_engines)]
                        s_eng_i += 1
                        eng.tensor_scalar(
                            out=S[:], in0=iota_sb[:],
                            scalar1=dloc_sb[:, ch:ch + 1],
                            scalar2=val_sb[:, ch:ch + 1],
                            op0=mybir.AluOpType.is_equal,
                            op1=mybir.AluOpType.mult,
                        )
                        nc.tensor.matmul(
                            out=acc[:], lhsT=S[:], rhs=g[:, j, 0:F],
                            start=(first[b] == ch), stop=(last[b] == ch),
                        )
                        if last[b] == ch:
                            epilogue(b, acc)
                            del alive[b]

            # ---------------- pass 1: T1 = L x ----------------
            def epi1(b, acc):
                stb = stagep.tile([P, F1], bf, name="st1", tag="st1")
                nc.scalar.activation(stb[:], acc[:],
                                     mybir.ActivationFunctionType.Copy)
                nc.scalar.dma_start(t1b_in[b * P:(b + 1) * P, :], stb[:])
                pt = dpsump.tile([P, P], bf, tag="dps", space="PSUM")
                nc.tensor.transpose(pt[:], stb[:], Ibf_sb[:])
                nc.vector.tensor_copy(t1T_sb[:, b * P:(b + 1) * P], pt[:])

            with nc.named_scope("pass1"):
                spmm("t1", None, F1, epi1, callsA, metaA, idxA_sb,
                     dlocA_sb, valA_sb, firstA, lastA, src_copy=t_xg,
                     s_engines=[nc.vector, nc.vector, nc.gpsimd])

            if phases >= 2:
                with nc.named_scope("ag_t1"):
                    allgather(nc, t1b_in, t1_full)

            # ------- pass 2: U = L T1, fused dense -------
            def epi2(b, acc):
                ns = slice(b * P, (b + 1) * P)
                ub = stagep.tile([P, F1], bf, name="ub", tag="ub")
                nc.scalar.activation(ub[:], acc[:],
                                     mybir.ActivationFunctionType.Copy)
                ptu = dpsump.tile([P, P], bf, tag="dps", space="PSUM")
                nc.tensor.transpose(ptu[:], ub[:], Ibf_sb[:])
                uT = densep.tile([P, P], bf, tag="uT")
                nc.vector.tensor_copy(uT[:], ptu[:])
                hT = []
                for half in range(nkc):
                    cs = slice(half * P, (half + 1) * P)
                    ph = dpsump.tile([P, P], f32, tag="dps", space="PSUM")
                    nc.tensor.matmul(ph[:], lhsT=A_sb[0][:, cs],
                                     rhs=xT_sb[:, ns], start=True, stop=False)
                    nc.tensor.matmul(ph[:], lhsT=A_sb[1][:, cs],
                                     rhs=t1T_sb[:, ns], start=False,
                                     stop=False)
                    nc.tensor.matmul(ph[:], lhsT=A_sb[2][:, cs],
                                     rhs=uT[:], start=False, stop=True)
                    h = densep.tile([P, P], bf, name=f"hT{half}",
                                    tag=f"hT{half}")
                    nc.scalar.activation(h[:], ph[:],
                                         mybir.ActivationFunctionType.Relu,
                                         bias=b1_sb[half][:])
                    hT.append(h)
                # ab = [a | 2b] feature-major -> node-major
                pab = dpsump.tile([P, P], f32, tag="dps", space="PSUM")
                for k in range(nkc):
                    nc.tensor.matmul(pab[:], lhsT=Ck_sb[k][:, 0:2 * FO],
                                     rhs=hT[k][:], start=(k == 0),
                                     stop=(k == nkc - 1))
                abf = stagep.tile([P, P], bf, name="abf", tag="abf")
                nc.scalar.activation(abf[:], pab[:],
                                     mybir.ActivationFunctionType.Copy)
                pta = dpsump.tile([P, P], bf, tag="dps", space="PSUM")
                nc.tensor.transpose(pta[:], abf[:], Ibf_sb[:])
                abn = stagep.tile([P, P], bf, name="abn", tag="abn")
                nc.vector.tensor_copy(abn[:], pta[:])
                nc.scalar.dma_start(ab_in[ns, :], abn[:])
                # c = h @ (W2_0 - W2_2) + b2
                pc_ = dpsump.tile([FO, P], f32, tag="dps", space="PSUM")
                for k in range(nkc):
                    nc.tensor.matmul(pc_[:], lhsT=Ck_sb[k][:, 2 * FO:3 * FO],
                                     rhs=hT[k][:], start=(k == 0),
                                     stop=(k == nkc - 1))
                cf = densep.tile([FO, P], f32, tag="cf")
                nc.vector.tensor_scalar(out=cf[:], in0=pc_[:],
                                        scalar1=b2_sb[:], scalar2=None,
                                        op0=mybir.AluOpType.add)
                ptc = dpsump.tile([P, FO], f32, tag="dps", space="PSUM")
                nc.tensor.transpose(ptc[:], cf[:], I64_sb[:])
                cn = densep.tile([P, FO], f32, tag="cn")
                nc.vector.tensor_copy(cn[:], ptc[:])
                nc.scalar.dma_start(c_in[ns, :], cn[:])

            if phases >= 3:
                with nc.named_scope("pass2"):
                    spmm("u", t1_full[:], F1, epi2, callsA, metaA, idxA_sb,
                         dlocA_sb, valA_sb, firstA, lastA,
                         s_engines=[nc.vector] * 5 + [nc.gpsimd])

            if phases >= 4:
                with nc.named_scope("ag_ab"):
                    allgather(nc, ab_in, ab_full)
                # gather c rows into pass-4 slot order (overlaps pass 3)
                nc.gpsimd.dma_gather(
                    out_ap=cg_sb[:], in_ap=c_in[:],
                    idxs_ap=cidx_sb[:, 0:NB4 * 8],
                    num_idxs=NB4 * P, num_idxs_reg=NB4 * P,
                    elem_size=FO, single_packet=False,
                )

            # ------- pass 3: s = a + 2Lb -------
            def epi3(b, acc):
                asb = stagep.tile([P, FO], bf, name="asb", tag="asb")
                nc.sync.dma_start(asb[:], ab_in[b * P:(b + 1) * P, 0:FO])
                st = stagep.tile([P, FO], bf, name="st3", tag="st3")
                nc.vector.tensor_tensor(out=st[:], in0=acc[:, FO:2 * FO],
                                        in1=asb[:],
                                        op=mybir.AluOpType.add)
                nc.scalar.dma_start(s_in[b * P:(b + 1) * P, 0:FO], st[:])

            if phases >= 4:
                with nc.named_scope("pass3"):
                    spmm("s", ab_full[:], F1, epi3, callsA, metaA, idxA_sb,
                         dlocA_sb, valA_sb, firstA, lastA,
                         s_engines=[nc.vector] * 5 + [nc.gpsimd])

            if phases >= 5:
                with nc.named_scope("ag_s"):
                    allgather(nc, s_in, s_full)

            # ------- pass 4: out = c + L s (pruned) -------
            def epi4(b, acc):
                st = stagep.tile([P, FO], f32, name="st4", tag="st4")
                nc.vector.tensor_tensor(out=st[:], in0=acc[:],
                                        in1=cg_sb[:, b, :],
                                        op=mybir.AluOpType.add)
                nc.scalar.dma_start(t_out.ap()[b * P:(b + 1) * P, :], st[:])

            if phases >= 5:
                with nc.named_scope("pass4"):
                    spmm("o", s_full[:], FO, epi4, callsB, metaB, None,
                         dlocB_sb, valB_sb, firstB, lastB, idx_dram=t_idxB,
                         s_engines=[nc.vector] * 5 + [nc.gpsimd])

            if dbg:
                def cvt(dst, src, F):
                    for t in range(0, SHP, P):
                        tl = stagep.tile([P, F], f32, tag=f"dbg{F}")
                        sb = stagep.tile([P, F], bf, tag=f"dbgb{F}")
                        nc.sync.dma_start(sb[:], src[t:t + P, 0:F])
                        nc.vector.tensor_copy(tl[:], sb[:])
                        nc.sync.dma_start(dst[t:t + P, 0:F], tl[:])
                cvt(t_dbg_t1.ap(), t1b_in, F1)
                if phases >= 3:
                    cvt(t_dbg_ab.ap(), ab_in, F1)
                    nc.sync.dma_start(t_dbg_c.ap(), c_in[:])
                if phases >= 4:
                    cvt(t_dbg_s.ap(), s_in, FO)

    nc.compile()
    return nc


# --------------------------------------------------------------------------
# Entry point
# --------------------------------------------------------------------------

def run(x, vals, W1, b1, W2, b2, rows, cols, idx=None, trace=False,
        debug=False):
    dims, shared, cores = host_prep(x, vals, W1, b1, W2, b2, rows, cols,
                                    idx=idx)
    if debug:
        dims["debug"] = True
    nc = build_program(dims)
    in_maps = []
    for c in range(NCORES):
        m = dict(
            A=shared["A"], Ck=shared["Ck"],
            b1h=shared["b1h"], b2h=shared["b2h"], iota=shared["iota"],
            Ibf=shared["Ibf"], I64=shared["I64"],
            xTb=cores[c]["xTb"], idxA=cores[c]["idxA"],
            xg=cores[c]["xg"],
            dlocA=cores[c]["dlocA"], valA=cores[c]["valA"],
            idxB=cores[c]["idxB"], dlocB=cores[c]["dlocB"],
            valB=cores[c]["valB"], cidx=cores[c]["cidx"],
        )
        in_maps.append(m)
    try:
        res = bass_utils.run_bass_kernel_spmd(
            nc, in_maps, core_ids=list(range(NCORES)), trace=trace)
    except Exception:
        res = bass_utils.run_bass_kernel_spmd(
            nc, in_maps, core_ids=list(range(NCORES)), trace=trace)
    SH = dims["SH"]
    N = dims["N"]
    FO = dims["FO"]
    out_full = np.zeros((N, FO), np.float32)
    for c in range(NCORES):
        co = cores[c]
        out_full[c * SH + co["d4"]] = res.results[c]["out2"][co["jpos4"]]
    return out_full, res


def kernel(x, vals, W1, b1, W2, b2, rows, cols, idx):
    out_full, _ = run(np.asarray(x), np.asarray(vals), np.asarray(W1),
                      np.asarray(b1), np.asarray(W2), np.asarray(b2),
                      np.asarray(rows), np.asarray(cols),
                      idx=np.asarray(idx))
    return out_full[np.asarray(idx)]
